# revision 10
# baseline (speedup 1.0000x reference)
"""GroupTopK (DeepSeek noaux-tc MoE routing) Trainium2 Bass kernel.

Contract: kernel(**inputs) takes FULL unsharded inputs
(scores [131072,256] f32, correction_bias [256] f32, scalars) and returns
(topk_weights [131072,8] f32, topk_ids [131072,8] i32), matching reference().

Strategy: token-parallel across 8 NeuronCores (16384 tokens each). Per
128-token tile on device, work is split across engines so the DVE (the
bottleneck) only runs ops it alone can do:
  ACT   : s = sigmoid(x)
  GPSIMD: sb = s + bias            (tensor_tensor add, [128,256])
  DVE   : per-group top8 (8x max8) -> group top2-sums (strided add) ->
          group threshold (max8) -> additive mask madd in {0,-BIG}
          (one fused tensor_scalar) -> masked per-group top8s (TT add)
          -> top8 values vb (max8)
  GPSIMD: mf = sb + madd_bcast     (masked full row)
  DVE   : ids = max_index(vb, mf)  (ties break low-index like jax.lax.top_k)
Input DMA is batched 4 token-tiles per dma_start; outputs are staged in
SBUF and written with one DMA per tensor at the end.
"""

from contextlib import ExitStack

import numpy as np

import concourse.bacc as bacc
import concourse.bass as bass
import concourse.mybir as mybir
import concourse.tile as tile
from concourse.alu_op_type import AluOpType
from concourse.bass_utils import run_bass_kernel_spmd

F32 = mybir.dt.float32
U32 = mybir.dt.uint32

BIG = 1e30
ACT = mybir.ActivationFunctionType

N_CORES = 8
T_FULL = 131072
E, G, GS = 256, 8, 32
DMA_BATCH = 4


def _build_program(
    T_core: int,
    scaling_factor: float,
    repeat: int = 1,
    bufs=(3, 4, 4),
    small_on_pool=False,
    rank_thresh=False,
):
    assert T_core % (128 * DMA_BATCH) == 0
    NT = T_core // 128

    nc = bacc.Bacc(
        "TRN2", target_bir_lowering=False, debug=False, enable_partition_id=False
    )
    x_d = nc.dram_tensor("scores", [T_core, E], F32, kind="ExternalInput")
    bb_d = nc.dram_tensor("bias_bcast", [128, E], F32, kind="ExternalInput")
    w_d = nc.dram_tensor("w_out", [128, NT * 8], F32, kind="ExternalOutput")
    id_d = nc.dram_tensor("id_out", [128, NT * 8], U32, kind="ExternalOutput")

    # Batched input view: DMA batch b covers token rows
    # [128*DMA_BATCH*b, 128*DMA_BATCH*(b+1)); partition p holds tokens
    # {base + 128*c + p : c in 0..DMA_BATCH-1} as free-dim chunks.
    xv = x_d[:, :].rearrange("(n c p) e -> n p c e", p=128, c=DMA_BATCH)

    with ExitStack() as ctx:
        tc = ctx.enter_context(tile.TileContext(nc))
        const_pool = ctx.enter_context(tc.tile_pool(name="const", bufs=1))
        bias_t = const_pool.tile([128, E], F32)
        nc.sync.dma_start(bias_t[:, :], bb_d[:, :])
        outw_t = const_pool.tile([128, NT * 8], F32)
        outi_t = const_pool.tile([128, NT * 8], U32)

        xin = ctx.enter_context(tc.tile_pool(name="xin", bufs=bufs[0]))
        work = ctx.enter_context(tc.tile_pool(name="work", bufs=bufs[1]))
        small = ctx.enter_context(tc.tile_pool(name="small", bufs=bufs[2]))

        for _rep in range(repeat):
            for b in range(NT // DMA_BATCH):
                xt = xin.tile([128, DMA_BATCH * E], F32, tag="x")
                nc.gpsimd.dma_start(
                    xt[:, :].rearrange("p (c e) -> p c e", c=DMA_BATCH), xv[b]
                )
                for c in range(DMA_BATCH):
                    n = b * DMA_BATCH + c
                    xs = xt[:, c * E : (c + 1) * E]

                    s_t = work.tile([128, E], F32, tag="s")
                    nc.scalar.activation(s_t[:, :], xs, ACT.Sigmoid)

                    sb_t = work.tile([128, E], F32, tag="sb")
                    nc.gpsimd.tensor_tensor(
                        sb_t[:, :], s_t[:, :], bias_t[:, :], op=AluOpType.add
                    )

                    g8 = small.tile([128, 64], F32, tag="g8")
                    for g in range(G):
                        nc.vector.max(
                            g8[:, 8 * g : 8 * g + 8], sb_t[:, GS * g : GS * (g + 1)]
                        )
                    g8v = g8[:, :].rearrange("p (g r) -> p g r", g=G)

                    gsc = small.tile([128, 8], F32, tag="gsc")
                    eng_sm = nc.gpsimd if small_on_pool else nc.vector
                    eng_sm.tensor_tensor(
                        gsc[:, :], g8v[:, :, 0], g8v[:, :, 1], op=AluOpType.add
                    )

                    madd = small.tile([128, 8], F32, tag="madd")
                    if rank_thresh:
                        # rank_i = #{j: gsc_j > gsc_i}; selected iff rank <= 3.
                        # Same over-selection on exact score ties as the
                        # 4th-largest threshold variant below.
                        cmp = small.tile([128, 64], F32, tag="cmp")
                        nc.gpsimd.tensor_tensor(
                            cmp[:, :].rearrange("p (i j) -> p i j", i=G),
                            gsc[:, :].broadcast_to([128, G, 8]),
                            gsc[:, :].rearrange("p (a j) -> p a j", a=1)
                            .broadcast_to([128, G, 8]),
                            op=AluOpType.is_lt,
                        )
                        rank = small.tile([128, 8], F32, tag="rank")
                        nc.gpsimd.tensor_reduce(
                            rank[:, :],
                            cmp[:, :].rearrange("p (i j) -> p i j", i=G),
                            axis=mybir.AxisListType.X,
                            op=AluOpType.add,
                        )
                        nc.gpsimd.tensor_scalar(
                            madd[:, :], rank[:, :], 3.5, -BIG,
                            op0=AluOpType.is_gt, op1=AluOpType.mult,
                        )
                    else:
                        gsort = small.tile([128, 8], F32, tag="gsort")
                        nc.vector.max(gsort[:, :], gsc[:, :])
                        # madd = (gsc < 4th-largest) * -BIG -> {0 sel, -BIG not}
                        eng_sm.tensor_scalar(
                            madd[:, :], gsc[:, :], gsort[:, 3:4], -BIG,
                            op0=AluOpType.is_lt, op1=AluOpType.mult,
                        )

                    g8m = small.tile([128, 64], F32, tag="g8m")
                    eng_sm.tensor_tensor(
                        g8m[:, :].rearrange("p (g r) -> p g r", g=G),
                        g8v,
                        madd[:, :].broadcast_to([128, G, 8]),
                        op=AluOpType.add,
                    )
                    vb_slice = outw_t[:, n * 8 : (n + 1) * 8]
                    nc.vector.max(vb_slice, g8m[:, :])

                    mf = work.tile([128, E], F32, tag="mf")
                    nc.gpsimd.tensor_tensor(
                        mf[:, :].rearrange("p (g e) -> p g e", g=G),
                        sb_t[:, :].rearrange("p (g e) -> p g e", g=G),
                        madd[:, :].broadcast_to([128, G, GS]),
                        op=AluOpType.add,
                    )

                    ids_slice = outi_t[:, n * 8 : (n + 1) * 8]
                    nc.vector.max_index(ids_slice, vb_slice, mf[:, :])

        nc.gpsimd.dma_start(w_d[:, :], outw_t[:, :])
        nc.gpsimd.dma_start(id_d[:, :], outi_t[:, :])

    nc.compile()
    return nc


_CACHE = {}


def _get_program(T_core: int, scaling_factor: float, repeat: int = 1):
    key = (T_core, float(scaling_factor), repeat)
    if key not in _CACHE:
        _CACHE[key] = _build_program(
            T_core, scaling_factor, repeat, small_on_pool=True
        )
    return _CACHE[key]


def _aux_inputs(bias: np.ndarray):
    return np.ascontiguousarray(np.broadcast_to(bias.astype(np.float32), (128, E)))


def kernel(
    scores,
    correction_bias,
    routed_scaling_factor,
    n_group,
    topk_group,
    topk,
    renormalize,
    _trace=False,
):
    scores = np.asarray(scores, dtype=np.float32)
    bias = np.asarray(correction_bias, dtype=np.float32)
    rsf = float(np.asarray(routed_scaling_factor))
    assert int(n_group) == G and int(topk_group) == 4
    assert int(topk) == 8 and int(renormalize) == 1

    T = scores.shape[0]
    T_core = T // N_CORES
    nc = _get_program(T_core, rsf)
    bias_bcast = _aux_inputs(bias)

    in_maps = []
    for i in range(N_CORES):
        in_maps.append(
            {
                "scores": np.ascontiguousarray(
                    scores[i * T_core : (i + 1) * T_core]
                ),
                "bias_bcast": bias_bcast,
            }
        )

    res = run_bass_kernel_spmd(
        nc, in_maps, core_ids=list(range(N_CORES)), trace=_trace
    )

    NT = T_core // 128
    vbs, ids = [], []
    for r in res.results:
        v = r["w_out"].reshape(128, NT, 8).transpose(1, 0, 2).reshape(T_core, 8)
        i_ = (
            r["id_out"]
            .view(np.int32)
            .reshape(128, NT, 8)
            .transpose(1, 0, 2)
            .reshape(T_core, 8)
        )
        vbs.append(v)
        ids.append(i_)
    vb = np.concatenate(vbs, 0)
    topk_ids = np.concatenate(ids, 0)

    # Unshard epilogue: the device returns the top-8 *biased* gate values
    # (vb = sigmoid(x) + bias at the selected experts, in top-k order) plus
    # the expert ids. The device ACT sigmoid can differ from the reference
    # f32 sigmoid by ~1ulp, which may swap adjacent near-tied entries
    # within the selected 8; re-rank the 8 with an f32-exact key
    # (stable sort, ties break toward lower expert id like jax.lax.top_k).
    x_at = np.take_along_axis(scores, topk_ids, axis=1).astype(np.float32)
    try:
        import jax

        s_h = np.asarray(jax.nn.sigmoid(x_at), dtype=np.float32)
    except Exception:
        s_h = 1.0 / (1.0 + np.exp(-x_at, dtype=np.float32))
    sb_h = s_h + bias[topk_ids]
    order = np.argsort(-sb_h, axis=1, kind="stable")
    s = np.take_along_axis(vb - bias[topk_ids], order, axis=1)
    topk_ids = np.ascontiguousarray(np.take_along_axis(topk_ids, order, axis=1))
    topk_weights = np.ascontiguousarray(
        (s / (s.sum(-1, keepdims=True) + 1e-20) * rsf).astype(np.float32)
    )
    if _trace:
        kernel.last_exec_time_ns = res.exec_time_ns
    return topk_weights, topk_ids


# revision 14
# speedup vs baseline: 1.0196x; 1.0196x over previous
"""GroupTopK (DeepSeek noaux-tc MoE routing) Trainium2 Bass kernel.

Contract: kernel(**inputs) takes FULL unsharded inputs
(scores [131072,256] f32, correction_bias [256] f32, scalars) and returns
(topk_weights [131072,8] f32, topk_ids [131072,8] i32), matching reference().

Strategy: token-parallel across 8 NeuronCores (16384 tokens each). Per
128-token tile on device, work is split across engines so the DVE (the
bottleneck) only runs the ops it alone can do (default config:
small_on_pool=True, mf_on_pool=True):
  ACT   : s = sigmoid(x)
  GPSIMD: sb = s + bias (TT add) ; group top2-sums gsc (strided add) ;
          additive mask madd = (gsc < thr)*-BIG (fused tensor_scalar) ;
          masked per-group top8s g8m (TT add) ; mf = sb + madd_bcast
  DVE   : per-group top8 (8x max8) -> group threshold thr (max8 on gsc)
          -> top8 values vb (max8 on g8m)
          -> ids = max_index(vb, mf)  (ties break low-index like top_k)
Per-tile engine-busy (TRN2 cost model): DVE 1275 ns (bottleneck, ~100%
occupied in CoreSim), GPSIMD ~900 ns, ACT ~440 ns; ~173 us/core at
16384 tokens vs ~290 us for the all-DVE baseline. Input DMA is batched
4 token-tiles per dma_start; outputs are staged in SBUF and written
with one DMA per tensor at the end.
"""

from contextlib import ExitStack

import numpy as np

import concourse.bacc as bacc
import concourse.bass as bass
import concourse.mybir as mybir
import concourse.tile as tile
from concourse.alu_op_type import AluOpType
from concourse.bass_utils import run_bass_kernel_spmd

F32 = mybir.dt.float32
U32 = mybir.dt.uint32

BIG = 1e30
ACT = mybir.ActivationFunctionType

N_CORES = 8
T_FULL = 131072
E, G, GS = 256, 8, 32
DMA_BATCH = 4


def _build_program(
    T_core: int,
    scaling_factor: float,
    repeat: int = 1,
    bufs=(3, 4, 4),
    small_on_pool=False,
    mf_on_pool=True,
):
    assert T_core % (128 * DMA_BATCH) == 0
    NT = T_core // 128

    nc = bacc.Bacc(
        "TRN2", target_bir_lowering=False, debug=False, enable_partition_id=False
    )
    x_d = nc.dram_tensor("scores", [T_core, E], F32, kind="ExternalInput")
    bb_d = nc.dram_tensor("bias_bcast", [128, E], F32, kind="ExternalInput")
    w_d = nc.dram_tensor("w_out", [128, NT * 8], F32, kind="ExternalOutput")
    id_d = nc.dram_tensor("id_out", [128, NT * 8], U32, kind="ExternalOutput")

    # Batched input view: DMA batch b covers token rows
    # [128*DMA_BATCH*b, 128*DMA_BATCH*(b+1)); partition p holds tokens
    # {base + 128*c + p : c in 0..DMA_BATCH-1} as free-dim chunks.
    xv = x_d[:, :].rearrange("(n c p) e -> n p c e", p=128, c=DMA_BATCH)

    with ExitStack() as ctx:
        tc = ctx.enter_context(tile.TileContext(nc))
        const_pool = ctx.enter_context(tc.tile_pool(name="const", bufs=1))
        bias_t = const_pool.tile([128, E], F32)
        nc.sync.dma_start(bias_t[:, :], bb_d[:, :])
        outw_t = const_pool.tile([128, NT * 8], F32)
        outi_t = const_pool.tile([128, NT * 8], U32)

        xin = ctx.enter_context(tc.tile_pool(name="xin", bufs=bufs[0]))
        work = ctx.enter_context(tc.tile_pool(name="work", bufs=bufs[1]))
        small = ctx.enter_context(tc.tile_pool(name="small", bufs=bufs[2]))

        for _rep in range(repeat):
            for b in range(NT // DMA_BATCH):
                xt = xin.tile([128, DMA_BATCH * E], F32, tag="x")
                nc.gpsimd.dma_start(
                    xt[:, :].rearrange("p (c e) -> p c e", c=DMA_BATCH), xv[b]
                )
                for c in range(DMA_BATCH):
                    n = b * DMA_BATCH + c
                    xs = xt[:, c * E : (c + 1) * E]

                    s_t = work.tile([128, E], F32, tag="s")
                    nc.scalar.activation(s_t[:, :], xs, ACT.Sigmoid)

                    sb_t = work.tile([128, E], F32, tag="sb")
                    nc.gpsimd.tensor_tensor(
                        sb_t[:, :], s_t[:, :], bias_t[:, :], op=AluOpType.add
                    )

                    g8 = small.tile([128, 64], F32, tag="g8")
                    for g in range(G):
                        nc.vector.max(
                            g8[:, 8 * g : 8 * g + 8], sb_t[:, GS * g : GS * (g + 1)]
                        )
                    g8v = g8[:, :].rearrange("p (g r) -> p g r", g=G)

                    gsc = small.tile([128, 8], F32, tag="gsc")
                    eng_sm = nc.gpsimd if small_on_pool else nc.vector
                    eng_sm.tensor_tensor(
                        gsc[:, :], g8v[:, :, 0], g8v[:, :, 1], op=AluOpType.add
                    )

                    gsort = small.tile([128, 8], F32, tag="gsort")
                    nc.vector.max(gsort[:, :], gsc[:, :])

                    # madd = (gsc < 4th-largest) * -BIG -> {0 sel, -BIG not}
                    madd = small.tile([128, 8], F32, tag="madd")
                    eng_sm.tensor_scalar(
                        madd[:, :], gsc[:, :], gsort[:, 3:4], -BIG,
                        op0=AluOpType.is_lt, op1=AluOpType.mult,
                    )

                    g8m = small.tile([128, 64], F32, tag="g8m")
                    eng_sm.tensor_tensor(
                        g8m[:, :].rearrange("p (g r) -> p g r", g=G),
                        g8v,
                        madd[:, :].broadcast_to([128, G, 8]),
                        op=AluOpType.add,
                    )
                    vb_slice = outw_t[:, n * 8 : (n + 1) * 8]
                    nc.vector.max(vb_slice, g8m[:, :])

                    mf = work.tile([128, E], F32, tag="mf")
                    eng_mf = nc.gpsimd if mf_on_pool else nc.vector
                    eng_mf.tensor_tensor(
                        mf[:, :].rearrange("p (g e) -> p g e", g=G),
                        sb_t[:, :].rearrange("p (g e) -> p g e", g=G),
                        madd[:, :].broadcast_to([128, G, GS]),
                        op=AluOpType.add,
                    )

                    ids_slice = outi_t[:, n * 8 : (n + 1) * 8]
                    nc.vector.max_index(ids_slice, vb_slice, mf[:, :])

        nc.gpsimd.dma_start(w_d[:, :], outw_t[:, :])
        nc.gpsimd.dma_start(id_d[:, :], outi_t[:, :])

    nc.compile()
    return nc


_CACHE = {}


def _get_program(T_core: int, scaling_factor: float, repeat: int = 1):
    key = (T_core, float(scaling_factor), repeat)
    if key not in _CACHE:
        _CACHE[key] = _build_program(
            T_core, scaling_factor, repeat, small_on_pool=True
        )
    return _CACHE[key]


def _aux_inputs(bias: np.ndarray):
    return np.ascontiguousarray(np.broadcast_to(bias.astype(np.float32), (128, E)))


def kernel(
    scores,
    correction_bias,
    routed_scaling_factor,
    n_group,
    topk_group,
    topk,
    renormalize,
    _trace=False,
):
    scores = np.asarray(scores, dtype=np.float32)
    bias = np.asarray(correction_bias, dtype=np.float32)
    rsf = float(np.asarray(routed_scaling_factor))
    assert int(n_group) == G and int(topk_group) == 4
    assert int(topk) == 8 and int(renormalize) == 1

    T = scores.shape[0]
    T_core = T // N_CORES
    nc = _get_program(T_core, rsf)
    bias_bcast = _aux_inputs(bias)

    in_maps = []
    for i in range(N_CORES):
        in_maps.append(
            {
                "scores": np.ascontiguousarray(
                    scores[i * T_core : (i + 1) * T_core]
                ),
                "bias_bcast": bias_bcast,
            }
        )

    res = run_bass_kernel_spmd(
        nc, in_maps, core_ids=list(range(N_CORES)), trace=_trace
    )

    NT = T_core // 128
    vbs, ids = [], []
    for r in res.results:
        v = r["w_out"].reshape(128, NT, 8).transpose(1, 0, 2).reshape(T_core, 8)
        i_ = (
            r["id_out"]
            .view(np.int32)
            .reshape(128, NT, 8)
            .transpose(1, 0, 2)
            .reshape(T_core, 8)
        )
        vbs.append(v)
        ids.append(i_)
    vb = np.concatenate(vbs, 0)
    topk_ids = np.concatenate(ids, 0)

    # Unshard epilogue: the device returns the top-8 *biased* gate values
    # (vb = sigmoid(x) + bias at the selected experts, in top-k order) plus
    # the expert ids. The device ACT sigmoid can differ from the reference
    # f32 sigmoid by ~1ulp, which may swap adjacent near-tied entries
    # within the selected 8; re-rank the 8 with an f32-exact key
    # (stable sort, ties break toward lower expert id like jax.lax.top_k).
    x_at = np.take_along_axis(scores, topk_ids, axis=1).astype(np.float32)
    try:
        import jax

        s_h = np.asarray(jax.nn.sigmoid(x_at), dtype=np.float32)
    except Exception:
        s_h = 1.0 / (1.0 + np.exp(-x_at, dtype=np.float32))
    sb_h = s_h + bias[topk_ids]
    order = np.argsort(-sb_h, axis=1, kind="stable")
    s = np.take_along_axis(vb - bias[topk_ids], order, axis=1)
    topk_ids = np.ascontiguousarray(np.take_along_axis(topk_ids, order, axis=1))
    topk_weights = np.ascontiguousarray(
        (s / (s.sum(-1, keepdims=True) + 1e-20) * rsf).astype(np.float32)
    )
    if _trace:
        kernel.last_exec_time_ns = res.exec_time_ns
    return topk_weights, topk_ids
